# revision 6
# baseline (speedup 1.0000x reference)
"""Multi-head attention + residual + LayerNorm kernel for Trainium2 (8 NeuronCores).

Sharding: pure data parallel over batch (B=8 -> 1 batch element per core).
No collectives. All heavy matmuls run in fp8e4 DoubleRow mode (K packed in
pairs of 128-partition chunks -> [128, 2, free] tiles, 2 MACs/cell/cycle),
with fp32 PSUM accumulation. Softmax internals (mask-add, exp input) and the
residual/LayerNorm tail stay bf16/fp32, so the end-to-end rel err vs the
fp32 reference is ~2e-3.

Per-core dataflow per head h:
  qT_h[dh,S]  = Wq_h^T @ Qb^T     (DR fp8; evict ACT copy -> fp8 pair tiles)
  kT_h[dh,S]  = Wk_h^T @ Kb^T     (DR fp8)
  v_h [S,dh]  = Vb @ Wv_h         (DR fp8; DVE evict)
  ST  [Sk,Sq] = kT_h^T @ qT_h     (DR fp8, transposed scores: Sk on partitions)
  tmp = ST/sqrt(dh) + maskT       (DVE scalar_tensor_tensor, bf16)
  E   = exp(tmp)                  (ACT, stored fp8 pair tiles)
  denb[128,Sq] = ones^T @ E       (DR fp8; every row = column sum of E)
  dcol[Sq-chunk,1] per chunk via f32 matmul with ones/128 (row->col transpose)
  recip = 1/dcol                  (DVE [128,8] f32)
  ctxT[dh,Sq] = v_h^T @ E         (DR fp8, unnormalized, stored fp8 pairs)
  outp[Sq,D] += (ctxT^T @ Wo_h) * recip  (DR fp8 + fused DVE STT eviction;
                                  h==0 seeds the accumulator with residual Q)
Finally LayerNorm over D per row: bn_stats/bn_aggr + sqrt + reciprocal.
"""

import sys

sys.path.insert(0, "/opt/trn_rl_repo")

import numpy as np
import ml_dtypes

B, S, D, H = 8, 1024, 512, 8
DH = 2 * D            # per-head dim (module uses d_model*2 per head)
P = 128               # SBUF partitions
NS = S // 512         # 512-wide free-dim chunks over sequence (2)
MT = S // P           # 128-partition tiles over sequence (8)
KD = D // P           # 128-chunks over d_model (4)
KH = DH // P          # 128-chunks over per-head dim (8)
JD = KD // 2          # DoubleRow K-pairs over d_model (2)
JH = KH // 2          # DoubleRow K-pairs over per-head dim (4)
SCALE = 1.0 / float(np.sqrt(DH))
NEG = -30000.0        # additive mask value (exp -> 0)

_cache = {}


def build_nc(repeat=1, nonzero_bias=False, nonzero_affine=False):
    """Build the per-core Bass program. All 8 cores run this SPMD."""
    import concourse.bass as bass
    import concourse.tile as tile
    from concourse import bacc, mybir

    f32 = mybir.dt.float32
    bf16 = mybir.dt.bfloat16
    f8 = mybir.dt.float8e4
    AF = mybir.ActivationFunctionType
    OP = mybir.AluOpType
    DR = mybir.MatmulPerfMode.DoubleRow

    nc = bacc.Bacc("TRN2", target_bir_lowering=False, debug=False, num_devices=8)

    # DRAM I/O (per core). Pair layouts: [j, 128(p), 2(par), cols] where the
    # contraction index is k = 2*j + par, row = k*128 + p.
    xq_d = nc.dram_tensor("xq", [JD, P, 2, S], f8, kind="ExternalInput").ap()
    xk_d = nc.dram_tensor("xk", [JD, P, 2, S], f8, kind="ExternalInput").ap()
    xv_d = nc.dram_tensor("xv", [JD, P, 2, S], f8, kind="ExternalInput").ap()
    qres = nc.dram_tensor("qres", [S, D], f32, kind="ExternalInput").ap()
    maskT = nc.dram_tensor("maskT", [MT // 2, P, 2, S], bf16, kind="ExternalInput").ap()
    wq_d = nc.dram_tensor("wq", [JD, P, 2, H * DH], f8, kind="ExternalInput").ap()
    wk_d = nc.dram_tensor("wk", [JD, P, 2, H * DH], f8, kind="ExternalInput").ap()
    wv_d = nc.dram_tensor("wv", [JD, P, 2, H * DH], f8, kind="ExternalInput").ap()
    wo_d = nc.dram_tensor("wo", [H, JH, P, 2, D], f8, kind="ExternalInput").ap()
    if nonzero_bias:
        bq = nc.dram_tensor("bq", [H * DH], f32, kind="ExternalInput").ap()
        bk = nc.dram_tensor("bk", [H * DH], f32, kind="ExternalInput").ap()
        bv = nc.dram_tensor("bv", [H * DH], f32, kind="ExternalInput").ap()
        bo = nc.dram_tensor("bo", [D], f32, kind="ExternalInput").ap()
    if nonzero_affine:
        gam = nc.dram_tensor("gam", [D], f32, kind="ExternalInput").ap()
        bet = nc.dram_tensor("bet", [D], f32, kind="ExternalInput").ap()
    out = nc.dram_tensor("out", [S, D], f32, kind="ExternalOutput").ap()

    def bcast_ap(src_1d, n):
        return bass.AP(tensor=src_1d.tensor, offset=src_1d.offset,
                       ap=[[0, P]] + list(src_1d.ap))

    with tile.TileContext(nc) as tc:
        import contextlib
        with contextlib.ExitStack() as ctx:
            const = ctx.enter_context(tc.tile_pool(name="const", bufs=1))
            persist = ctx.enter_context(tc.tile_pool(name="persist", bufs=1))
            wpool = ctx.enter_context(tc.tile_pool(name="wpool", bufs=8))
            wopool = ctx.enter_context(tc.tile_pool(name="wopool", bufs=8))
            qt_pool = ctx.enter_context(tc.tile_pool(name="qt", bufs=6))
            kt_pool = ctx.enter_context(tc.tile_pool(name="kt", bufs=6))
            v_pool = ctx.enter_context(tc.tile_pool(name="vv", bufs=6))
            e_pool = ctx.enter_context(tc.tile_pool(name="ee", bufs=6))
            cx_pool = ctx.enter_context(tc.tile_pool(name="cx", bufs=6))
            tmp_pool = ctx.enter_context(tc.tile_pool(name="tmp", bufs=4))
            den_pool = ctx.enter_context(tc.tile_pool(name="den", bufs=2))
            recip_pool = ctx.enter_context(tc.tile_pool(name="recip", bufs=2))
            resid_pool = ctx.enter_context(tc.tile_pool(name="resid", bufs=8))
            stat_pool = ctx.enter_context(tc.tile_pool(name="stat", bufs=8))
            mm_psum = ctx.enter_context(tc.tile_pool(name="mmps", bufs=3, space="PSUM"))
            out_psum = ctx.enter_context(tc.tile_pool(name="ops", bufs=2, space="PSUM"))

            ones_pair = const.tile([P, 2, P], f8)
            nc.vector.memset(ones_pair, 1.0)
            ones_inv = const.tile([P, 1], f32)
            nc.vector.memset(ones_inv, 1.0 / P)
            eps_t = const.tile([P, 1], f32)
            nc.vector.memset(eps_t, 1e-5)

            if nonzero_bias:
                bq_col = const.tile([P, H * KH], f32)
                nc.sync.dma_start(bq_col, bq.rearrange("(c p) -> p c", p=P))
                bk_col = const.tile([P, H * KH], f32)
                nc.sync.dma_start(bk_col, bk.rearrange("(c p) -> p c", p=P))
                bo_b = const.tile([P, D], f32)
                nc.sync.dma_start(bo_b, bcast_ap(bo, D))
            if nonzero_affine:
                gam_b = const.tile([P, D], f32)
                nc.sync.dma_start(gam_b, bcast_ap(gam, D))
                bet_b = const.tile([P, D], f32)
                nc.sync.dma_start(bet_b, bcast_ap(bet, D))

            # Persistent SBUF inputs (fp8 pair tiles)
            xq_sb = [persist.tile([P, 2, S], f8, tag=f"xq{j}", name=f"xq{j}") for j in range(JD)]
            xk_sb = [persist.tile([P, 2, S], f8, tag=f"xk{j}", name=f"xk{j}") for j in range(JD)]
            xv_sb = [persist.tile([P, 2, S], f8, tag=f"xv{j}", name=f"xv{j}") for j in range(JD)]
            mask_sb = [persist.tile([P, 2, S], bf16, tag=f"mk{m}", name=f"mk{m}") for m in range(MT // 2)]
            acc_sb = [persist.tile([P, D], f32, tag=f"ac{m}", name=f"ac{m}") for m in range(MT)]

            def proj_evict_act(dst, ps, col):
                # PSUM -> fp8 pair tile slice on ScalarE, with optional bias
                if nonzero_bias:
                    nc.scalar.activation(dst, ps, AF.Identity, bias=col)
                else:
                    nc.scalar.copy(dst, ps)

            def body(iv=None):
                wq0 = [wpool.tile([P, 2, DH], f8, tag="w", name="wq0") for _ in range(JD)]
                wk0 = [wpool.tile([P, 2, DH], f8, tag="w", name="wk0") for _ in range(JD)]
                for j in range(JD):
                    nc.sync.dma_start(xq_sb[j], xq_d[j])
                    nc.sync.dma_start(wq0[j], wq_d[j, :, :, 0:DH])
                for j in range(JD):
                    nc.sync.dma_start(xk_sb[j], xk_d[j])
                    nc.sync.dma_start(wk0[j], wk_d[j, :, :, 0:DH])
                for j in range(JD):
                    nc.sync.dma_start(xv_sb[j], xv_d[j])
                for m in range(MT // 2):
                    nc.sync.dma_start(mask_sb[m], maskT[m])
                qres_sb = [resid_pool.tile([P, D], f32, tag="resid", name="qres_sb")
                           for _ in range(MT)]
                for m in range(MT):
                    nc.sync.dma_start(qres_sb[m], qres[m * P:(m + 1) * P, :])

                for h in range(H):
                    hof = h * DH
                    if h == 0:
                        wq_t, wk_t = wq0, wk0
                    else:
                        wq_t = [wpool.tile([P, 2, DH], f8, tag="w", name="wq_t") for _ in range(JD)]
                        wk_t = [wpool.tile([P, 2, DH], f8, tag="w", name="wk_t") for _ in range(JD)]
                        for j in range(JD):
                            nc.sync.dma_start(wq_t[j], wq_d[j, :, :, hof:hof + DH])
                            nc.sync.dma_start(wk_t[j], wk_d[j, :, :, hof:hof + DH])

                    # ---- qT_h / kT_h projections [DH, S] (DoubleRow over K=D)
                    qt = [qt_pool.tile([P, 2, S], f8, tag="qt", name="qt") for _ in range(JH)]
                    kt = [kt_pool.tile([P, 2, S], f8, tag="kt", name="kt") for _ in range(JH)]
                    for w_t, x_sb, dst, bcol in ((wq_t, xq_sb, qt, "bq"),
                                                 (wk_t, xk_sb, kt, "bk")):
                        for m in range(KH):
                            ps = mm_psum.tile([P, S], f32, tag="mm", name="ps")
                            for j in range(JD):
                                for n in range(NS):
                                    nc.tensor.matmul(
                                        ps[:, n * 512:(n + 1) * 512],
                                        lhsT=w_t[j][:, :, m * P:(m + 1) * P],
                                        rhs=x_sb[j][:, :, n * 512:(n + 1) * 512],
                                        start=(j == 0), stop=(j == JD - 1),
                                        perf_mode=DR, skip_group_check=True)
                            col = None
                            if nonzero_bias:
                                cidx = h * KH + m
                                col = (bq_col if bcol == "bq" else bk_col)[:, cidx:cidx + 1]
                            proj_evict_act(dst[m // 2][:, m % 2, :], ps, col)

                    # ---- v projection weights + optional bias broadcast
                    wv_t = [wpool.tile([P, 2, DH], f8, tag="w", name="wv_t") for _ in range(JD)]
                    for j in range(JD):
                        nc.sync.dma_start(wv_t[j], wv_d[j, :, :, hof:hof + DH])
                    if nonzero_bias:
                        bv_b = recip_pool.tile([P, DH], f32, tag="bvb", bufs=2, name="bv_b")
                        nc.sync.dma_start(bv_b, bcast_ap(bv[hof:hof + DH], DH))

                    # ---- scores (transposed) + mask + exp -> E fp8 pairs
                    est = [e_pool.tile([P, 2, S], f8, tag="e", name="est") for _ in range(JH)]
                    for mp in range(MT // 2):
                        tmp = tmp_pool.tile([P, 2, S], bf16, tag="tmp", name="tmp")
                        for par in range(2):
                            m = 2 * mp + par
                            ps = mm_psum.tile([P, S], f32, tag="mm", name="ps")
                            for j in range(JH):
                                for n in range(NS):
                                    nc.tensor.matmul(
                                        ps[:, n * 512:(n + 1) * 512],
                                        lhsT=kt[j][:, :, m * P:(m + 1) * P],
                                        rhs=qt[j][:, :, n * 512:(n + 1) * 512],
                                        start=(j == 0), stop=(j == JH - 1),
                                        perf_mode=DR, skip_group_check=True)
                            nc.vector.scalar_tensor_tensor(
                                out=tmp[:, par, :], in0=ps, scalar=SCALE,
                                in1=mask_sb[mp][:, par, :],
                                op0=OP.mult, op1=OP.add)
                        nc.scalar.activation(est[mp], tmp, AF.Exp)

                    # ---- v projection [Sk, DH] (PE work overlapping exp)
                    vt = [v_pool.tile([P, 2, DH], f8, tag="v", name="vt") for _ in range(JH)]
                    for m in range(MT):
                        ps = mm_psum.tile([P, DH], f32, tag="mm", name="ps")
                        for j in range(JD):
                            for n in range(NS):
                                nc.tensor.matmul(
                                    ps[:, n * 512:(n + 1) * 512],
                                    lhsT=xv_sb[j][:, :, m * P:(m + 1) * P],
                                    rhs=wv_t[j][:, :, n * 512:(n + 1) * 512],
                                    start=(j == 0), stop=(j == JD - 1),
                                    perf_mode=DR, skip_group_check=True)
                        dst = vt[m // 2][:, m % 2, :]
                        if nonzero_bias:
                            nc.vector.tensor_add(dst, ps, bv_b)
                        else:
                            nc.vector.tensor_copy(dst, ps)

                    # ---- denominator via ones-matmul, then row->col transpose
                    den_sb = den_pool.tile([P, S], f32, tag="den", name="den_sb")
                    ps = mm_psum.tile([P, S], f32, tag="mm", name="ps")
                    for n in range(NS):
                        for j in range(JH):
                            nc.tensor.matmul(
                                ps[:, n * 512:(n + 1) * 512], lhsT=ones_pair,
                                rhs=est[j][:, :, n * 512:(n + 1) * 512],
                                start=(j == 0), stop=(j == JH - 1),
                                perf_mode=DR, skip_group_check=True)
                    nc.scalar.copy(den_sb, ps)
                    dcol = out_psum.tile([P, D], f32, tag="op", name="dcol")
                    for m in range(MT):
                        nc.tensor.matmul(dcol[:, m:m + 1],
                                         lhsT=den_sb[:, m * P:(m + 1) * P],
                                         rhs=ones_inv, start=True, stop=True)
                    recip = recip_pool.tile([P, MT], f32, tag="recip", name="recip")
                    nc.vector.reciprocal(recip, dcol[:, 0:MT])

                    # ---- wo prefetch (fp8 pairs)
                    wo_t = [wopool.tile([P, 2, D], f8, tag="wo", name="wo_t") for _ in range(JH)]
                    for j in range(JH):
                        nc.sync.dma_start(wo_t[j], wo_d[h, j])

                    # ---- ctxT[DH, S] unnormalized (DoubleRow over K=S)
                    cxt = [cx_pool.tile([P, 2, S], f8, tag="cx", name="cxt") for _ in range(JH)]
                    for m in range(KH):
                        ps = mm_psum.tile([P, S], f32, tag="mm", name="ps")
                        for j in range(JH):
                            for n in range(NS):
                                nc.tensor.matmul(
                                    ps[:, n * 512:(n + 1) * 512],
                                    lhsT=vt[j][:, :, m * P:(m + 1) * P],
                                    rhs=est[j][:, :, n * 512:(n + 1) * 512],
                                    start=(j == 0), stop=(j == JH - 1),
                                    perf_mode=DR, skip_group_check=True)
                        nc.vector.tensor_copy(cxt[m // 2][:, m % 2, :], ps)

                    # ---- output projection + normalize + accumulate
                    for m in range(MT):
                        ps = out_psum.tile([P, D], f32, tag="op", name="ops")
                        for j in range(JH):
                            nc.tensor.matmul(
                                ps, lhsT=cxt[j][:, :, m * P:(m + 1) * P],
                                rhs=wo_t[j], start=(j == 0), stop=(j == JH - 1),
                                perf_mode=DR)
                        seed = qres_sb[m] if h == 0 else acc_sb[m]
                        nc.vector.scalar_tensor_tensor(
                            out=acc_sb[m], in0=ps, scalar=recip[:, m:m + 1],
                            in1=seed, op0=OP.mult, op1=OP.add)

                # ---- residual bias + LayerNorm + store
                for m in range(MT):
                    x = acc_sb[m]
                    if nonzero_bias:
                        nc.vector.tensor_add(x, x, bo_b)
                    st = stat_pool.tile([P, 6], f32, tag="st", name="st")
                    nc.vector.bn_stats(st, x)
                    mv = stat_pool.tile([P, 2], f32, tag="mv", name="mv")
                    nc.vector.bn_aggr(mv, st)
                    std = stat_pool.tile([P, 1], f32, tag="sd", name="std")
                    nc.scalar.activation(std, mv[:, 1:2], AF.Sqrt, bias=eps_t)
                    rstd = stat_pool.tile([P, 1], f32, tag="rs", name="rstd")
                    nc.vector.reciprocal(rstd, std)
                    y = resid_pool.tile([P, D], f32, tag="resid", name="y")
                    nc.vector.tensor_scalar(
                        out=y, in0=x, scalar1=mv[:, 0:1], scalar2=rstd,
                        op0=OP.subtract, op1=OP.mult)
                    if nonzero_affine:
                        nc.vector.tensor_mul(y, y, gam_b)
                        nc.vector.tensor_add(y, y, bet_b)
                    nc.sync.dma_start(out[m * P:(m + 1) * P, :], y)

            if repeat == 1:
                body()
            else:
                with tc.For_i(0, repeat, 1) as iv:
                    body(iv)

    nc.compile()
    return nc




_LDW_DEDUP_STATS = {"dropped": 0, "total": 0}


def _dedup_ldw_json(bir_json: bytes) -> bytes:
    """Drop PE Ldweights that reload the exact weights already resident.

    The preceding identical Ldweights in the same block's PE stream leaves the
    stationary operand in place; Matmult(ldweights=false) then reuses it. Sync
    waits/updates from a dropped Ldweights are merged into the next kept PE
    instruction (later in the same engine queue, so ordering is preserved).
    """
    import json as _json
    d = _json.loads(bir_json)
    marker = False
    for fn in d.get("functions", []):
        for al in fn.get("allocations", []):
            if isinstance(al, dict) and al.get("name", "").startswith("maskT"):
                marker = True
    if not marker:
        return bir_json
    dropped = total = 0
    for fn in d["functions"]:
        for blk in fn.get("blocks", []):
            insts = blk.get("instructions", [])
            keep = []
            last_key = None
            pending = None
            for inst in insts:
                if inst.get("engine") != "PE":
                    keep.append(inst)
                    continue
                op = inst.get("opcode")
                if op == "Ldweights":
                    total += 1
                    key = _json.dumps(
                        [inst.get("ins"), inst.get("tile_position"),
                         inst.get("tile_size"), inst.get("perf_mode"),
                         inst.get("is_transpose")], sort_keys=True)
                    if key == last_key:
                        si = inst.get("sync_info")
                        if si and (si.get("on_wait") or si.get("on_update")):
                            if pending is None:
                                pending = {"on_wait": [], "on_update": []}
                            pending["on_wait"].extend(si.get("on_wait") or [])
                            pending["on_update"].extend(si.get("on_update") or [])
                        dropped += 1
                        continue
                    last_key = key
                else:
                    if op != "Matmult":
                        # unknown PE op: conservatively forget weight state
                        last_key = None
                if pending is not None:
                    si = inst.setdefault("sync_info", {"on_wait": [], "on_update": []})
                    si.setdefault("on_wait", []).extend(pending["on_wait"])
                    si.setdefault("on_update", []).extend(pending["on_update"])
                    pending = None
                keep.append(inst)
            assert pending is None, "dangling sync from dropped Ldweights"
            blk["instructions"] = keep
    _LDW_DEDUP_STATS["dropped"] += dropped
    _LDW_DEDUP_STATS["total"] += total
    return _json.dumps(d).encode()


def _install_ldw_dedup():
    import concourse.bass_utils as _bu
    import concourse.bass2jax as _b2j
    if getattr(_bu, "_ldw_dedup_installed", False):
        return
    _orig = _bu.compile_bir_kernel

    def _patched(bir_json, tmpdir, neff_name="file.neff"):
        try:
            bir_json = _dedup_ldw_json(bir_json)
        except Exception:
            pass
        return _orig(bir_json, tmpdir, neff_name)

    _bu.compile_bir_kernel = _patched
    _b2j.compile_bir_kernel = _patched
    _bu._ldw_dedup_installed = True


def _pack_pairs_rows(a):
    """[K*128, C] -> [K//2, 128, 2, C] pair layout (k = 2*j + par, row = k*128+p)."""
    K = a.shape[0] // P
    return np.ascontiguousarray(
        a.reshape(K // 2, 2, P, a.shape[1]).transpose(0, 2, 1, 3))


def _prep_in_maps(inputs):
    f8 = ml_dtypes.float8_e4m3
    Q = np.asarray(inputs["Q"], np.float32)
    K = np.asarray(inputs["K"], np.float32)
    V = np.asarray(inputs["V"], np.float32)
    mask = np.asarray(inputs["attn_mask"])
    wq = _pack_pairs_rows(np.asarray(inputs["Wq"], np.float32)).astype(f8)
    wk = _pack_pairs_rows(np.asarray(inputs["Wk"], np.float32)).astype(f8)
    wv = _pack_pairs_rows(np.asarray(inputs["Wv"], np.float32)).astype(f8)
    # Wo: [H*DH, D] -> [H, JH, 128, 2, D]
    wo = np.asarray(inputs["Wo"], np.float32).reshape(H, JH, 2, P, D)
    wo = np.ascontiguousarray(wo.transpose(0, 1, 3, 2, 4)).astype(f8)

    nonzero_bias = any(np.any(np.asarray(inputs[k])) for k in ("bq", "bk", "bv", "bo"))
    nonzero_affine = (np.any(np.asarray(inputs["gamma"]) != 1.0)
                      or np.any(np.asarray(inputs["beta"])))

    in_maps = []
    for b in range(B):
        m = {
            "xq": _pack_pairs_rows(np.ascontiguousarray(Q[b].T)).astype(f8),
            "xk": _pack_pairs_rows(np.ascontiguousarray(K[b].T)).astype(f8),
            "xv": _pack_pairs_rows(np.ascontiguousarray(V[b].T)).astype(f8),
            "qres": np.ascontiguousarray(Q[b]),
            "maskT": _pack_pairs_rows(np.ascontiguousarray(
                np.where(mask[b].T, np.float32(NEG), np.float32(0))
            )).astype(ml_dtypes.bfloat16),
            "wq": wq, "wk": wk, "wv": wv, "wo": wo,
        }
        if nonzero_bias:
            m["bq"] = np.asarray(inputs["bq"], np.float32)
            m["bk"] = np.asarray(inputs["bk"], np.float32)
            m["bv"] = np.asarray(inputs["bv"], np.float32)
            m["bo"] = np.asarray(inputs["bo"], np.float32)
        if nonzero_affine:
            m["gam"] = np.asarray(inputs["gamma"], np.float32)
            m["bet"] = np.asarray(inputs["beta"], np.float32)
        in_maps.append(m)
    return in_maps, nonzero_bias, nonzero_affine


def kernel(**inputs):
    from concourse.bass_utils import run_bass_kernel_spmd

    _install_ldw_dedup()

    in_maps, nzb, nza = _prep_in_maps(inputs)
    key = (1, nzb, nza)
    if key not in _cache:
        _cache[key] = build_nc(repeat=1, nonzero_bias=nzb, nonzero_affine=nza)
    nc = _cache[key]
    res = run_bass_kernel_spmd(nc, in_maps, list(range(B)))
    return np.stack([res.results[c]["out"] for c in range(B)], axis=0).astype(np.float32)


# revision 8
# speedup vs baseline: 1.1218x; 1.1218x over previous
"""Multi-head attention + residual + LayerNorm kernel for Trainium2 (8 NeuronCores).

Sharding: pure data parallel over batch (B=8 -> 1 batch element per core).
No collectives. All heavy matmuls run in fp8e4 DoubleRow mode (K packed in
pairs of 128-partition chunks -> [128, 2, free] tiles, 2 MACs/cell/cycle),
with fp32 PSUM accumulation. Softmax internals (mask-add, exp input) and the
residual/LayerNorm tail stay bf16/fp32, so the end-to-end rel err vs the
fp32 reference is ~2e-3.

Per-core dataflow per head h:
  qT_h[dh,S]  = Wq_h^T @ Qb^T     (DR fp8; evict ACT copy -> fp8 pair tiles)
  kT_h[dh,S]  = Wk_h^T @ Kb^T     (DR fp8)
  v_h [S,dh]  = Vb @ Wv_h         (DR fp8; DVE evict)
  ST  [Sk,Sq] = kT_h^T @ qT_h     (DR fp8, transposed scores: Sk on partitions)
  tmp = ST/sqrt(dh) + maskT       (DVE scalar_tensor_tensor, bf16)
  E   = exp(tmp)                  (ACT, stored fp8 pair tiles)
  denb[128,Sq] = ones^T @ E       (DR fp8; every row = column sum of E)
  dcol[Sq-chunk,1] per chunk via f32 matmul with ones/128 (row->col transpose)
  recip = 1/dcol                  (DVE [128,8] f32)
  ctxT[dh,Sq] = v_h^T @ E         (DR fp8, unnormalized, stored fp8 pairs)
  outp[Sq,D] += (ctxT^T @ Wo_h) * recip  (DR fp8 + fused DVE STT eviction;
                                  h==0 seeds the accumulator with residual Q)
Finally LayerNorm over D per row: bn_stats/bn_aggr + sqrt + reciprocal.
"""

import sys

sys.path.insert(0, "/opt/trn_rl_repo")

import numpy as np
import ml_dtypes

B, S, D, H = 8, 1024, 512, 8
DH = 2 * D            # per-head dim (module uses d_model*2 per head)
P = 128               # SBUF partitions
NS = S // 512         # 512-wide free-dim chunks over sequence (2)
MT = S // P           # 128-partition tiles over sequence (8)
KD = D // P           # 128-chunks over d_model (4)
KH = DH // P          # 128-chunks over per-head dim (8)
JD = KD // 2          # DoubleRow K-pairs over d_model (2)
JH = KH // 2          # DoubleRow K-pairs over per-head dim (4)
SCALE = 1.0 / float(np.sqrt(DH))
NEG = -30000.0        # additive mask value (exp -> 0)

_cache = {}


def build_nc(repeat=1, nonzero_bias=False, nonzero_affine=False):
    """Build the per-core Bass program. All 8 cores run this SPMD."""
    import concourse.bass as bass
    import concourse.tile as tile
    from concourse import bacc, mybir

    f32 = mybir.dt.float32
    bf16 = mybir.dt.bfloat16
    f8 = mybir.dt.float8e4
    AF = mybir.ActivationFunctionType
    OP = mybir.AluOpType
    DR = mybir.MatmulPerfMode.DoubleRow

    nc = bacc.Bacc("TRN2", target_bir_lowering=False, debug=False, num_devices=8)

    # DRAM I/O (per core). Pair layouts: [j, 128(p), 2(par), cols] where the
    # contraction index is k = 2*j + par, row = k*128 + p.
    xq_d = nc.dram_tensor("xq", [JD, P, 2, S], f8, kind="ExternalInput").ap()
    xk_d = nc.dram_tensor("xk", [JD, P, 2, S], f8, kind="ExternalInput").ap()
    xv_d = nc.dram_tensor("xv", [JD, P, 2, S], f8, kind="ExternalInput").ap()
    qres = nc.dram_tensor("qres", [S, D], f32, kind="ExternalInput").ap()
    maskT = nc.dram_tensor("maskT", [MT // 2, P, 2, S], bf16, kind="ExternalInput").ap()
    wq_d = nc.dram_tensor("wq", [JD, P, 2, H * DH], f8, kind="ExternalInput").ap()
    wk_d = nc.dram_tensor("wk", [JD, P, 2, H * DH], f8, kind="ExternalInput").ap()
    wv_d = nc.dram_tensor("wv", [JD, P, 2, H * DH], f8, kind="ExternalInput").ap()
    wo_d = nc.dram_tensor("wo", [H, JH, P, 2, D], f8, kind="ExternalInput").ap()
    if nonzero_bias:
        bq = nc.dram_tensor("bq", [H * DH], f32, kind="ExternalInput").ap()
        bk = nc.dram_tensor("bk", [H * DH], f32, kind="ExternalInput").ap()
        bv = nc.dram_tensor("bv", [H * DH], f32, kind="ExternalInput").ap()
        bo = nc.dram_tensor("bo", [D], f32, kind="ExternalInput").ap()
    if nonzero_affine:
        gam = nc.dram_tensor("gam", [D], f32, kind="ExternalInput").ap()
        bet = nc.dram_tensor("bet", [D], f32, kind="ExternalInput").ap()
    out = nc.dram_tensor("out", [S, D], f32, kind="ExternalOutput").ap()

    def bcast_ap(src_1d, n):
        return bass.AP(tensor=src_1d.tensor, offset=src_1d.offset,
                       ap=[[0, P]] + list(src_1d.ap))

    with tile.TileContext(nc) as tc:
        import contextlib
        with contextlib.ExitStack() as ctx:
            const = ctx.enter_context(tc.tile_pool(name="const", bufs=1))
            persist = ctx.enter_context(tc.tile_pool(name="persist", bufs=1))
            wpool = ctx.enter_context(tc.tile_pool(name="wpool", bufs=8))
            wopool = ctx.enter_context(tc.tile_pool(name="wopool", bufs=8))
            qt_pool = ctx.enter_context(tc.tile_pool(name="qt", bufs=6))
            kt_pool = ctx.enter_context(tc.tile_pool(name="kt", bufs=6))
            v_pool = ctx.enter_context(tc.tile_pool(name="vv", bufs=6))
            e_pool = ctx.enter_context(tc.tile_pool(name="ee", bufs=6))
            cx_pool = ctx.enter_context(tc.tile_pool(name="cx", bufs=6))
            tmp_pool = ctx.enter_context(tc.tile_pool(name="tmp", bufs=4))
            den_pool = ctx.enter_context(tc.tile_pool(name="den", bufs=2))
            recip_pool = ctx.enter_context(tc.tile_pool(name="recip", bufs=2))
            resid_pool = ctx.enter_context(tc.tile_pool(name="resid", bufs=8))
            stat_pool = ctx.enter_context(tc.tile_pool(name="stat", bufs=8))
            mm_psum = ctx.enter_context(tc.tile_pool(name="mmps", bufs=3, space="PSUM"))
            out_psum = ctx.enter_context(tc.tile_pool(name="ops", bufs=2, space="PSUM"))

            ones_pair = const.tile([P, 2, P], f8)
            nc.vector.memset(ones_pair, 1.0)
            ones_inv = const.tile([P, 1], f32)
            nc.vector.memset(ones_inv, 1.0 / P)
            eps_t = const.tile([P, 1], f32)
            nc.vector.memset(eps_t, 1e-5)

            if nonzero_bias:
                bq_col = const.tile([P, H * KH], f32)
                nc.sync.dma_start(bq_col, bq.rearrange("(c p) -> p c", p=P))
                bk_col = const.tile([P, H * KH], f32)
                nc.sync.dma_start(bk_col, bk.rearrange("(c p) -> p c", p=P))
                bo_b = const.tile([P, D], f32)
                nc.sync.dma_start(bo_b, bcast_ap(bo, D))
            if nonzero_affine:
                gam_b = const.tile([P, D], f32)
                nc.sync.dma_start(gam_b, bcast_ap(gam, D))
                bet_b = const.tile([P, D], f32)
                nc.sync.dma_start(bet_b, bcast_ap(bet, D))

            # Persistent SBUF inputs (fp8 pair tiles)
            xq_sb = [persist.tile([P, 2, S], f8, tag=f"xq{j}", name=f"xq{j}") for j in range(JD)]
            xk_sb = [persist.tile([P, 2, S], f8, tag=f"xk{j}", name=f"xk{j}") for j in range(JD)]
            xv_sb = [persist.tile([P, 2, S], f8, tag=f"xv{j}", name=f"xv{j}") for j in range(JD)]
            mask_sb = [persist.tile([P, 2, S], bf16, tag=f"mk{m}", name=f"mk{m}") for m in range(MT // 2)]
            acc_sb = [persist.tile([P, D], f32, tag=f"ac{m}", name=f"ac{m}") for m in range(MT)]

            def proj_evict_act(dst, ps, col):
                # PSUM -> fp8 pair tile slice on ScalarE, with optional bias
                if nonzero_bias:
                    nc.scalar.activation(dst, ps, AF.Identity, bias=col)
                else:
                    nc.scalar.copy(dst, ps)

            def body(iv=None):
                wq0 = [wpool.tile([P, 2, DH], f8, tag="w", name="wq0") for _ in range(JD)]
                wk0 = [wpool.tile([P, 2, DH], f8, tag="w", name="wk0") for _ in range(JD)]
                for j in range(JD):
                    nc.sync.dma_start(xq_sb[j], xq_d[j])
                    nc.sync.dma_start(wq0[j], wq_d[j, :, :, 0:DH])
                for j in range(JD):
                    nc.sync.dma_start(xk_sb[j], xk_d[j])
                    nc.sync.dma_start(wk0[j], wk_d[j, :, :, 0:DH])
                for j in range(JD):
                    nc.sync.dma_start(xv_sb[j], xv_d[j])
                for m in range(MT // 2):
                    nc.sync.dma_start(mask_sb[m], maskT[m])
                qres_sb = [resid_pool.tile([P, D], f32, tag="resid", name="qres_sb")
                           for _ in range(MT)]
                for m in range(MT):
                    nc.sync.dma_start(qres_sb[m], qres[m * P:(m + 1) * P, :])

                for h in range(H):
                    hof = h * DH
                    if h == 0:
                        wq_t, wk_t = wq0, wk0
                    else:
                        wq_t = [wpool.tile([P, 2, DH], f8, tag="w", name="wq_t") for _ in range(JD)]
                        wk_t = [wpool.tile([P, 2, DH], f8, tag="w", name="wk_t") for _ in range(JD)]
                        for j in range(JD):
                            nc.sync.dma_start(wq_t[j], wq_d[j, :, :, hof:hof + DH])
                            nc.sync.dma_start(wk_t[j], wk_d[j, :, :, hof:hof + DH])

                    # ---- qT_h / kT_h projections [DH, S] (DoubleRow over K=D)
                    qt = [qt_pool.tile([P, 2, S], f8, tag="qt", name="qt") for _ in range(JH)]
                    kt = [kt_pool.tile([P, 2, S], f8, tag="kt", name="kt") for _ in range(JH)]
                    for w_t, x_sb, dst, bcol in ((wq_t, xq_sb, qt, "bq"),
                                                 (wk_t, xk_sb, kt, "bk")):
                        for m in range(KH):
                            ps = mm_psum.tile([P, S], f32, tag="mm", name="ps")
                            for j in range(JD):
                                for n in range(NS):
                                    nc.tensor.matmul(
                                        ps[:, n * 512:(n + 1) * 512],
                                        lhsT=w_t[j][:, :, m * P:(m + 1) * P],
                                        rhs=x_sb[j][:, :, n * 512:(n + 1) * 512],
                                        start=(j == 0), stop=(j == JD - 1),
                                        perf_mode=DR, skip_group_check=True)
                            col = None
                            if nonzero_bias:
                                cidx = h * KH + m
                                col = (bq_col if bcol == "bq" else bk_col)[:, cidx:cidx + 1]
                            proj_evict_act(dst[m // 2][:, m % 2, :], ps, col)

                    # ---- v projection weights + optional bias broadcast
                    wv_t = [wpool.tile([P, 2, DH], f8, tag="w", name="wv_t") for _ in range(JD)]
                    for j in range(JD):
                        nc.sync.dma_start(wv_t[j], wv_d[j, :, :, hof:hof + DH])
                    if nonzero_bias:
                        bv_b = recip_pool.tile([P, DH], f32, tag="bvb", bufs=2, name="bv_b")
                        nc.sync.dma_start(bv_b, bcast_ap(bv[hof:hof + DH], DH))

                    # ---- scores (transposed) + mask + exp -> E fp8 pairs
                    est = [e_pool.tile([P, 2, S], f8, tag="e", name="est") for _ in range(JH)]
                    for mp in range(MT // 2):
                        tmp = tmp_pool.tile([P, 2, S], bf16, tag="tmp", name="tmp")
                        for par in range(2):
                            m = 2 * mp + par
                            ps = mm_psum.tile([P, S], f32, tag="mm", name="ps")
                            for j in range(JH):
                                for n in range(NS):
                                    nc.tensor.matmul(
                                        ps[:, n * 512:(n + 1) * 512],
                                        lhsT=kt[j][:, :, m * P:(m + 1) * P],
                                        rhs=qt[j][:, :, n * 512:(n + 1) * 512],
                                        start=(j == 0), stop=(j == JH - 1),
                                        perf_mode=DR, skip_group_check=True)
                            nc.vector.scalar_tensor_tensor(
                                out=tmp[:, par, :], in0=ps, scalar=SCALE,
                                in1=mask_sb[mp][:, par, :],
                                op0=OP.mult, op1=OP.add)
                        nc.scalar.activation(est[mp], tmp, AF.Exp)

                    # ---- v projection [Sk, DH] (PE work overlapping exp)
                    vt = [v_pool.tile([P, 2, DH], f8, tag="v", name="vt") for _ in range(JH)]
                    for m in range(MT):
                        ps = mm_psum.tile([P, DH], f32, tag="mm", name="ps")
                        for j in range(JD):
                            for n in range(NS):
                                nc.tensor.matmul(
                                    ps[:, n * 512:(n + 1) * 512],
                                    lhsT=xv_sb[j][:, :, m * P:(m + 1) * P],
                                    rhs=wv_t[j][:, :, n * 512:(n + 1) * 512],
                                    start=(j == 0), stop=(j == JD - 1),
                                    perf_mode=DR, skip_group_check=True)
                        dst = vt[m // 2][:, m % 2, :]
                        if nonzero_bias:
                            nc.vector.tensor_add(dst, ps, bv_b)
                        else:
                            nc.vector.tensor_copy(dst, ps)

                    # ---- denominator via ones-matmul, then row->col transpose
                    den_sb = den_pool.tile([P, S], f32, tag="den", name="den_sb")
                    ps = mm_psum.tile([P, S], f32, tag="mm", name="ps")
                    for n in range(NS):
                        for j in range(JH):
                            nc.tensor.matmul(
                                ps[:, n * 512:(n + 1) * 512], lhsT=ones_pair,
                                rhs=est[j][:, :, n * 512:(n + 1) * 512],
                                start=(j == 0), stop=(j == JH - 1),
                                perf_mode=DR, skip_group_check=True)
                    nc.scalar.copy(den_sb, ps)
                    # ---- wo prefetch (fp8 pairs)
                    wo_t = [wopool.tile([P, 2, D], f8, tag="wo", name="wo_t") for _ in range(JH)]
                    for j in range(JH):
                        nc.sync.dma_start(wo_t[j], wo_d[h, j])

                    # ---- ctxT[DH, S] unnormalized (DoubleRow over K=S)
                    cxt = [cx_pool.tile([P, 2, S], f8, tag="cx", name="cxt") for _ in range(JH)]
                    for m in range(KH):
                        ps = mm_psum.tile([P, S], f32, tag="mm", name="ps")
                        for j in range(JH):
                            for n in range(NS):
                                nc.tensor.matmul(
                                    ps[:, n * 512:(n + 1) * 512],
                                    lhsT=vt[j][:, :, m * P:(m + 1) * P],
                                    rhs=est[j][:, :, n * 512:(n + 1) * 512],
                                    start=(j == 0), stop=(j == JH - 1),
                                    perf_mode=DR, skip_group_check=True)
                        nc.vector.tensor_copy(cxt[m // 2][:, m % 2, :], ps)

                    dcol = out_psum.tile([P, D], f32, tag="op", name="dcol")
                    for m in range(MT):
                        nc.tensor.matmul(dcol[:, m:m + 1],
                                         lhsT=den_sb[:, m * P:(m + 1) * P],
                                         rhs=ones_inv, start=True, stop=True)
                    recip = recip_pool.tile([P, MT], f32, tag="recip", name="recip")
                    nc.vector.reciprocal(recip, dcol[:, 0:MT])

                    # ---- output projection + normalize + accumulate
                    for m in range(MT):
                        ps = out_psum.tile([P, D], f32, tag="op", name="ops")
                        for j in range(JH):
                            nc.tensor.matmul(
                                ps, lhsT=cxt[j][:, :, m * P:(m + 1) * P],
                                rhs=wo_t[j], start=(j == 0), stop=(j == JH - 1),
                                perf_mode=DR)
                        seed = qres_sb[m] if h == 0 else acc_sb[m]
                        nc.vector.scalar_tensor_tensor(
                            out=acc_sb[m], in0=ps, scalar=recip[:, m:m + 1],
                            in1=seed, op0=OP.mult, op1=OP.add)

                # ---- residual bias + LayerNorm + store
                for m in range(MT):
                    x = acc_sb[m]
                    if nonzero_bias:
                        nc.vector.tensor_add(x, x, bo_b)
                    st = stat_pool.tile([P, 6], f32, tag="st", name="st")
                    nc.vector.bn_stats(st, x)
                    mv = stat_pool.tile([P, 2], f32, tag="mv", name="mv")
                    nc.vector.bn_aggr(mv, st)
                    std = stat_pool.tile([P, 1], f32, tag="sd", name="std")
                    nc.scalar.activation(std, mv[:, 1:2], AF.Sqrt, bias=eps_t)
                    rstd = stat_pool.tile([P, 1], f32, tag="rs", name="rstd")
                    nc.vector.reciprocal(rstd, std)
                    y = resid_pool.tile([P, D], f32, tag="resid", name="y")
                    nc.vector.tensor_scalar(
                        out=y, in0=x, scalar1=mv[:, 0:1], scalar2=rstd,
                        op0=OP.subtract, op1=OP.mult)
                    if nonzero_affine:
                        nc.vector.tensor_mul(y, y, gam_b)
                        nc.vector.tensor_add(y, y, bet_b)
                    nc.sync.dma_start(out[m * P:(m + 1) * P, :], y)

            if repeat == 1:
                body()
            else:
                with tc.For_i(0, repeat, 1) as iv:
                    body(iv)

    nc.compile()
    return nc




_LDW_DEDUP_STATS = {"dropped": 0, "total": 0}


def _dedup_ldw_json(bir_json: bytes) -> bytes:
    """Drop PE Ldweights that reload the exact weights already resident.

    The preceding identical Ldweights in the same block's PE stream leaves the
    stationary operand in place; Matmult(ldweights=false) then reuses it. Sync
    waits/updates from a dropped Ldweights are merged into the next kept PE
    instruction (later in the same engine queue, so ordering is preserved).
    """
    import json as _json
    d = _json.loads(bir_json)
    marker = False
    for fn in d.get("functions", []):
        for al in fn.get("allocations", []):
            if isinstance(al, dict) and al.get("name", "").startswith("maskT"):
                marker = True
    if not marker:
        return bir_json
    dropped = total = 0
    for fn in d["functions"]:
        for blk in fn.get("blocks", []):
            insts = blk.get("instructions", [])
            keep = []
            last_key = None
            pending = None
            for inst in insts:
                if inst.get("engine") != "PE":
                    keep.append(inst)
                    continue
                op = inst.get("opcode")
                if op == "Ldweights":
                    total += 1
                    key = _json.dumps(
                        [inst.get("ins"), inst.get("tile_position"),
                         inst.get("tile_size"), inst.get("perf_mode"),
                         inst.get("is_transpose")], sort_keys=True)
                    if key == last_key:
                        si = inst.get("sync_info")
                        if si and (si.get("on_wait") or si.get("on_update")):
                            if pending is None:
                                pending = {"on_wait": [], "on_update": []}
                            pending["on_wait"].extend(si.get("on_wait") or [])
                            pending["on_update"].extend(si.get("on_update") or [])
                        dropped += 1
                        continue
                    last_key = key
                else:
                    if op != "Matmult":
                        # unknown PE op: conservatively forget weight state
                        last_key = None
                if pending is not None:
                    si = inst.setdefault("sync_info", {"on_wait": [], "on_update": []})
                    si.setdefault("on_wait", []).extend(pending["on_wait"])
                    si.setdefault("on_update", []).extend(pending["on_update"])
                    pending = None
                keep.append(inst)
            assert pending is None, "dangling sync from dropped Ldweights"
            blk["instructions"] = keep
    _LDW_DEDUP_STATS["dropped"] += dropped
    _LDW_DEDUP_STATS["total"] += total
    return _json.dumps(d).encode()


def _install_ldw_dedup():
    import concourse.bass_utils as _bu
    import concourse.bass2jax as _b2j
    if getattr(_bu, "_ldw_dedup_installed", False):
        return
    _orig = _bu.compile_bir_kernel

    def _patched(bir_json, tmpdir, neff_name="file.neff"):
        try:
            bir_json = _dedup_ldw_json(bir_json)
        except Exception:
            pass
        return _orig(bir_json, tmpdir, neff_name)

    _bu.compile_bir_kernel = _patched
    _b2j.compile_bir_kernel = _patched
    _bu._ldw_dedup_installed = True


def _pack_pairs_rows(a):
    """[K*128, C] -> [K//2, 128, 2, C] pair layout (k = 2*j + par, row = k*128+p)."""
    K = a.shape[0] // P
    return np.ascontiguousarray(
        a.reshape(K // 2, 2, P, a.shape[1]).transpose(0, 2, 1, 3))


def _prep_in_maps(inputs):
    f8 = ml_dtypes.float8_e4m3
    Q = np.asarray(inputs["Q"], np.float32)
    K = np.asarray(inputs["K"], np.float32)
    V = np.asarray(inputs["V"], np.float32)
    mask = np.asarray(inputs["attn_mask"])
    wq = _pack_pairs_rows(np.asarray(inputs["Wq"], np.float32)).astype(f8)
    wk = _pack_pairs_rows(np.asarray(inputs["Wk"], np.float32)).astype(f8)
    wv = _pack_pairs_rows(np.asarray(inputs["Wv"], np.float32)).astype(f8)
    # Wo: [H*DH, D] -> [H, JH, 128, 2, D]
    wo = np.asarray(inputs["Wo"], np.float32).reshape(H, JH, 2, P, D)
    wo = np.ascontiguousarray(wo.transpose(0, 1, 3, 2, 4)).astype(f8)

    nonzero_bias = any(np.any(np.asarray(inputs[k])) for k in ("bq", "bk", "bv", "bo"))
    nonzero_affine = (np.any(np.asarray(inputs["gamma"]) != 1.0)
                      or np.any(np.asarray(inputs["beta"])))

    in_maps = []
    for b in range(B):
        m = {
            "xq": _pack_pairs_rows(np.ascontiguousarray(Q[b].T)).astype(f8),
            "xk": _pack_pairs_rows(np.ascontiguousarray(K[b].T)).astype(f8),
            "xv": _pack_pairs_rows(np.ascontiguousarray(V[b].T)).astype(f8),
            "qres": np.ascontiguousarray(Q[b]),
            "maskT": _pack_pairs_rows(np.ascontiguousarray(
                np.where(mask[b].T, np.float32(NEG), np.float32(0))
            )).astype(ml_dtypes.bfloat16),
            "wq": wq, "wk": wk, "wv": wv, "wo": wo,
        }
        if nonzero_bias:
            m["bq"] = np.asarray(inputs["bq"], np.float32)
            m["bk"] = np.asarray(inputs["bk"], np.float32)
            m["bv"] = np.asarray(inputs["bv"], np.float32)
            m["bo"] = np.asarray(inputs["bo"], np.float32)
        if nonzero_affine:
            m["gam"] = np.asarray(inputs["gamma"], np.float32)
            m["bet"] = np.asarray(inputs["beta"], np.float32)
        in_maps.append(m)
    return in_maps, nonzero_bias, nonzero_affine


def kernel(**inputs):
    from concourse.bass_utils import run_bass_kernel_spmd

    in_maps, nzb, nza = _prep_in_maps(inputs)
    key = (1, nzb, nza)
    if key not in _cache:
        _cache[key] = build_nc(repeat=1, nonzero_bias=nzb, nonzero_affine=nza)
    nc = _cache[key]
    res = run_bass_kernel_spmd(nc, in_maps, list(range(B)))
    return np.stack([res.results[c]["out"] for c in range(B)], axis=0).astype(np.float32)


# revision 9
# speedup vs baseline: 1.2177x; 1.0855x over previous
"""Multi-head attention + residual + LayerNorm kernel for Trainium2 (8 NeuronCores).

Sharding: pure data parallel over batch (B=8 -> 1 batch element per core).
No collectives. All heavy matmuls run in fp8e4 DoubleRow mode (K packed in
pairs of 128-partition chunks -> [128, 2, free] tiles, 2 MACs/cell/cycle),
with fp32 PSUM accumulation. Softmax internals (mask-add, exp input) and the
residual/LayerNorm tail stay bf16/fp32, so the end-to-end rel err vs the
fp32 reference is ~2e-3.

Per-core dataflow per head h:
  qT_h[dh,S]  = Wq_h^T @ Qb^T     (DR fp8; evict ACT copy -> fp8 pair tiles)
  kT_h[dh,S]  = Wk_h^T @ Kb^T     (DR fp8)
  v_h [S,dh]  = Vb @ Wv_h         (DR fp8; DVE evict)
  ST  [Sk,Sq] = kT_h^T @ qT_h     (DR fp8, transposed scores: Sk on partitions)
  tmp = ST/sqrt(dh) + maskT       (DVE scalar_tensor_tensor, bf16)
  E   = exp(tmp)                  (ACT, stored fp8 pair tiles)
  denb[128,Sq] = ones^T @ E       (DR fp8; every row = column sum of E)
  dcol[Sq-chunk,1] per chunk via f32 matmul with ones/128 (row->col transpose)
  recip = 1/dcol                  (DVE [128,8] f32)
  ctxT[dh,Sq] = v_h^T @ E         (DR fp8, unnormalized, stored fp8 pairs)
  outp[Sq,D] += (ctxT^T @ Wo_h) * recip  (DR fp8 + fused DVE STT eviction;
                                  h==0 seeds the accumulator with residual Q)
Finally LayerNorm over D per row: bn_stats/bn_aggr + sqrt + reciprocal.
"""

import sys

sys.path.insert(0, "/opt/trn_rl_repo")

import numpy as np
import ml_dtypes

B, S, D, H = 8, 1024, 512, 8
DH = 2 * D            # per-head dim (module uses d_model*2 per head)
P = 128               # SBUF partitions
NS = S // 512         # 512-wide free-dim chunks over sequence (2)
MT = S // P           # 128-partition tiles over sequence (8)
KD = D // P           # 128-chunks over d_model (4)
KH = DH // P          # 128-chunks over per-head dim (8)
JD = KD // 2          # DoubleRow K-pairs over d_model (2)
JH = KH // 2          # DoubleRow K-pairs over per-head dim (4)
SCALE = 1.0 / float(np.sqrt(DH))
NEG = -30000.0        # additive mask value (exp -> 0)

_cache = {}


def build_nc(repeat=1, nonzero_bias=False, nonzero_affine=False):
    """Build the per-core Bass program. All 8 cores run this SPMD."""
    import concourse.bass as bass
    import concourse.tile as tile
    from concourse import bacc, mybir

    f32 = mybir.dt.float32
    bf16 = mybir.dt.bfloat16
    f8 = mybir.dt.float8e4
    AF = mybir.ActivationFunctionType
    OP = mybir.AluOpType
    DR = mybir.MatmulPerfMode.DoubleRow

    nc = bacc.Bacc("TRN2", target_bir_lowering=False, debug=False, num_devices=8)

    # DRAM I/O (per core). Pair layouts: [j, 128(p), 2(par), cols] where the
    # contraction index is k = 2*j + par, row = k*128 + p.
    xq_d = nc.dram_tensor("xq", [JD, P, 2, S], f8, kind="ExternalInput").ap()
    xk_d = nc.dram_tensor("xk", [JD, P, 2, S], f8, kind="ExternalInput").ap()
    xv_d = nc.dram_tensor("xv", [JD, P, 2, S], f8, kind="ExternalInput").ap()
    qres = nc.dram_tensor("qres", [S, D], f32, kind="ExternalInput").ap()
    maskT = nc.dram_tensor("maskT", [MT // 2, P, 2, S], bf16, kind="ExternalInput").ap()
    wq_d = nc.dram_tensor("wq", [JD, P, 2, H * DH], f8, kind="ExternalInput").ap()
    wk_d = nc.dram_tensor("wk", [JD, P, 2, H * DH], f8, kind="ExternalInput").ap()
    wv_d = nc.dram_tensor("wv", [JD, P, 2, H * DH], f8, kind="ExternalInput").ap()
    wo_d = nc.dram_tensor("wo", [H, JH, P, 2, D], f8, kind="ExternalInput").ap()
    if nonzero_bias:
        bq = nc.dram_tensor("bq", [H * DH], f32, kind="ExternalInput").ap()
        bk = nc.dram_tensor("bk", [H * DH], f32, kind="ExternalInput").ap()
        bv = nc.dram_tensor("bv", [H * DH], f32, kind="ExternalInput").ap()
        bo = nc.dram_tensor("bo", [D], f32, kind="ExternalInput").ap()
    if nonzero_affine:
        gam = nc.dram_tensor("gam", [D], f32, kind="ExternalInput").ap()
        bet = nc.dram_tensor("bet", [D], f32, kind="ExternalInput").ap()
    out = nc.dram_tensor("out", [S, D], f32, kind="ExternalOutput").ap()

    def bcast_ap(src_1d, n):
        return bass.AP(tensor=src_1d.tensor, offset=src_1d.offset,
                       ap=[[0, P]] + list(src_1d.ap))

    with tile.TileContext(nc) as tc:
        import contextlib
        with contextlib.ExitStack() as ctx:
            const = ctx.enter_context(tc.tile_pool(name="const", bufs=1))
            persist = ctx.enter_context(tc.tile_pool(name="persist", bufs=1))
            wpool = ctx.enter_context(tc.tile_pool(name="wpool", bufs=8))
            wopool = ctx.enter_context(tc.tile_pool(name="wopool", bufs=8))
            qt_pool = ctx.enter_context(tc.tile_pool(name="qt", bufs=6))
            kt_pool = ctx.enter_context(tc.tile_pool(name="kt", bufs=6))
            v_pool = ctx.enter_context(tc.tile_pool(name="vv", bufs=6))
            e_pool = ctx.enter_context(tc.tile_pool(name="ee", bufs=6))
            cx_pool = ctx.enter_context(tc.tile_pool(name="cx", bufs=6))
            tmp_pool = ctx.enter_context(tc.tile_pool(name="tmp", bufs=4))
            den_pool = ctx.enter_context(tc.tile_pool(name="den", bufs=2))
            recip_pool = ctx.enter_context(tc.tile_pool(name="recip", bufs=2))
            resid_pool = ctx.enter_context(tc.tile_pool(name="resid", bufs=8))
            stat_pool = ctx.enter_context(tc.tile_pool(name="stat", bufs=8))
            mm_psum = ctx.enter_context(tc.tile_pool(name="mmps", bufs=3, space="PSUM"))
            out_psum = ctx.enter_context(tc.tile_pool(name="ops", bufs=2, space="PSUM"))

            ones_pair = const.tile([P, 2, P], f8)
            nc.vector.memset(ones_pair, 1.0)
            ones_inv = const.tile([P, 1], f32)
            nc.vector.memset(ones_inv, 1.0 / P)
            eps_t = const.tile([P, 1], f32)
            nc.vector.memset(eps_t, 1e-5)

            if nonzero_bias:
                bq_col = const.tile([P, H * KH], f32)
                nc.sync.dma_start(bq_col, bq.rearrange("(c p) -> p c", p=P))
                bk_col = const.tile([P, H * KH], f32)
                nc.sync.dma_start(bk_col, bk.rearrange("(c p) -> p c", p=P))
                bo_b = const.tile([P, D], f32)
                nc.sync.dma_start(bo_b, bcast_ap(bo, D))
            if nonzero_affine:
                gam_b = const.tile([P, D], f32)
                nc.sync.dma_start(gam_b, bcast_ap(gam, D))
                bet_b = const.tile([P, D], f32)
                nc.sync.dma_start(bet_b, bcast_ap(bet, D))

            # Persistent SBUF inputs (fp8 pair tiles)
            xq_sb = [persist.tile([P, 2, S], f8, tag=f"xq{j}", name=f"xq{j}") for j in range(JD)]
            xk_sb = [persist.tile([P, 2, S], f8, tag=f"xk{j}", name=f"xk{j}") for j in range(JD)]
            xv_sb = [persist.tile([P, 2, S], f8, tag=f"xv{j}", name=f"xv{j}") for j in range(JD)]
            mask_sb = [persist.tile([P, 2, S], bf16, tag=f"mk{m}", name=f"mk{m}") for m in range(MT // 2)]
            acc_sb = [persist.tile([P, D], f32, tag=f"ac{m}", name=f"ac{m}") for m in range(MT)]

            def proj_evict_act(dst, ps, col):
                # PSUM -> fp8 pair tile slice on ScalarE, with optional bias
                if nonzero_bias:
                    nc.scalar.activation(dst, ps, AF.Identity, bias=col)
                else:
                    nc.scalar.copy(dst, ps)

            def body(iv=None):
                wq0 = [wpool.tile([P, 2, DH], f8, tag="w", name="wq0") for _ in range(JD)]
                wk0 = [wpool.tile([P, 2, DH], f8, tag="w", name="wk0") for _ in range(JD)]
                for j in range(JD):
                    nc.sync.dma_start(xq_sb[j], xq_d[j])
                    nc.sync.dma_start(wq0[j], wq_d[j, :, :, 0:DH])
                for j in range(JD):
                    nc.sync.dma_start(xk_sb[j], xk_d[j])
                    nc.sync.dma_start(wk0[j], wk_d[j, :, :, 0:DH])
                for j in range(JD):
                    nc.sync.dma_start(xv_sb[j], xv_d[j])
                for m in range(MT // 2):
                    nc.sync.dma_start(mask_sb[m], maskT[m])
                qres_sb = [resid_pool.tile([P, D], f32, tag="resid", name="qres_sb")
                           for _ in range(MT)]
                for m in range(MT):
                    nc.sync.dma_start(qres_sb[m], qres[m * P:(m + 1) * P, :])

                for h in range(H):
                    hof = h * DH
                    if h == 0:
                        wq_t, wk_t = wq0, wk0
                    else:
                        wq_t = [wpool.tile([P, 2, DH], f8, tag="w", name="wq_t") for _ in range(JD)]
                        wk_t = [wpool.tile([P, 2, DH], f8, tag="w", name="wk_t") for _ in range(JD)]
                        for j in range(JD):
                            nc.sync.dma_start(wq_t[j], wq_d[j, :, :, hof:hof + DH])
                            nc.sync.dma_start(wk_t[j], wk_d[j, :, :, hof:hof + DH])

                    # ---- qT_h / kT_h projections [DH, S] (DoubleRow over K=D)
                    qt = [qt_pool.tile([P, 2, S], f8, tag="qt", name="qt") for _ in range(JH)]
                    kt = [kt_pool.tile([P, 2, S], f8, tag="kt", name="kt") for _ in range(JH)]
                    for w_t, x_sb, dst, bcol in ((wq_t, xq_sb, qt, "bq"),
                                                 (wk_t, xk_sb, kt, "bk")):
                        for m in range(KH):
                            ps = mm_psum.tile([P, S], f32, tag="mm", name="ps")
                            for j in range(JD):
                                for n in range(NS):
                                    nc.tensor.matmul(
                                        ps[:, n * 512:(n + 1) * 512],
                                        lhsT=w_t[j][:, :, m * P:(m + 1) * P],
                                        rhs=x_sb[j][:, :, n * 512:(n + 1) * 512],
                                        start=(j == 0), stop=(j == JD - 1),
                                        perf_mode=DR, skip_group_check=True)
                            col = None
                            if nonzero_bias:
                                cidx = h * KH + m
                                col = (bq_col if bcol == "bq" else bk_col)[:, cidx:cidx + 1]
                            proj_evict_act(dst[m // 2][:, m % 2, :], ps, col)

                    # ---- v projection weights + optional bias broadcast
                    wv_t = [wpool.tile([P, 2, DH], f8, tag="w", name="wv_t") for _ in range(JD)]
                    for j in range(JD):
                        nc.sync.dma_start(wv_t[j], wv_d[j, :, :, hof:hof + DH])
                    if nonzero_bias:
                        bv_b = recip_pool.tile([P, DH], f32, tag="bvb", bufs=2, name="bv_b")
                        nc.sync.dma_start(bv_b, bcast_ap(bv[hof:hof + DH], DH))

                    # ---- scores (transposed) + mask + exp -> E fp8 pairs
                    est = [e_pool.tile([P, 2, S], f8, tag="e", name="est") for _ in range(JH)]
                    for mp in range(MT // 2):
                        tmp = tmp_pool.tile([P, 2, S], bf16, tag="tmp", name="tmp")
                        for par in range(2):
                            m = 2 * mp + par
                            ps = mm_psum.tile([P, S], f32, tag="mm", name="ps")
                            for j in range(JH):
                                for n in range(NS):
                                    nc.tensor.matmul(
                                        ps[:, n * 512:(n + 1) * 512],
                                        lhsT=kt[j][:, :, m * P:(m + 1) * P],
                                        rhs=qt[j][:, :, n * 512:(n + 1) * 512],
                                        start=(j == 0), stop=(j == JH - 1),
                                        perf_mode=DR, skip_group_check=True)
                            nc.vector.scalar_tensor_tensor(
                                out=tmp[:, par, :], in0=ps, scalar=SCALE,
                                in1=mask_sb[mp][:, par, :],
                                op0=OP.mult, op1=OP.add)
                        nc.scalar.activation(est[mp], tmp, AF.Exp)

                    # ---- v projection [Sk, DH] (PE work overlapping exp)
                    vt = [v_pool.tile([P, 2, DH], f8, tag="v", name="vt") for _ in range(JH)]
                    for m in range(MT):
                        ps = mm_psum.tile([P, DH], f32, tag="mm", name="ps")
                        for j in range(JD):
                            for n in range(NS):
                                nc.tensor.matmul(
                                    ps[:, n * 512:(n + 1) * 512],
                                    lhsT=xv_sb[j][:, :, m * P:(m + 1) * P],
                                    rhs=wv_t[j][:, :, n * 512:(n + 1) * 512],
                                    start=(j == 0), stop=(j == JD - 1),
                                    perf_mode=DR, skip_group_check=True)
                        dst = vt[m // 2][:, m % 2, :]
                        if nonzero_bias:
                            nc.vector.tensor_add(dst, ps, bv_b)
                        else:
                            nc.vector.tensor_copy(dst, ps)

                    # ---- denominator via ones-matmul, then row->col transpose
                    den_sb = den_pool.tile([P, S], f32, tag="den", name="den_sb")
                    ps = mm_psum.tile([P, S], f32, tag="mm", name="ps")
                    for n in range(NS):
                        for j in range(JH):
                            nc.tensor.matmul(
                                ps[:, n * 512:(n + 1) * 512], lhsT=ones_pair,
                                rhs=est[j][:, :, n * 512:(n + 1) * 512],
                                start=(j == 0), stop=(j == JH - 1),
                                perf_mode=DR, skip_group_check=True)
                    nc.scalar.copy(den_sb, ps)
                    # ---- wo prefetch (fp8 pairs)
                    wo_t = [wopool.tile([P, 2, D], f8, tag="wo", name="wo_t") for _ in range(JH)]
                    for j in range(JH):
                        nc.sync.dma_start(wo_t[j], wo_d[h, j])

                    # ---- ctxT[DH, S] unnormalized (DoubleRow over K=S)
                    cxt = [cx_pool.tile([P, 2, S], f8, tag="cx", name="cxt") for _ in range(JH)]
                    for m in range(KH):
                        ps = mm_psum.tile([P, S], f32, tag="mm", name="ps")
                        for j in range(JH):
                            for n in range(NS):
                                nc.tensor.matmul(
                                    ps[:, n * 512:(n + 1) * 512],
                                    lhsT=vt[j][:, :, m * P:(m + 1) * P],
                                    rhs=est[j][:, :, n * 512:(n + 1) * 512],
                                    start=(j == 0), stop=(j == JH - 1),
                                    perf_mode=DR, skip_group_check=True)
                        nc.vector.tensor_copy(cxt[m // 2][:, m % 2, :], ps)

                    dcol = out_psum.tile([P, D], f32, tag="op", name="dcol")
                    for m in range(MT):
                        nc.tensor.matmul(dcol[:, m:m + 1],
                                         lhsT=den_sb[:, m * P:(m + 1) * P],
                                         rhs=ones_inv, start=True, stop=True)
                    recip = recip_pool.tile([P, MT], f32, tag="recip", name="recip")
                    nc.vector.reciprocal(recip, dcol[:, 0:MT])

                    # ---- output projection + normalize + accumulate
                    for m in range(MT):
                        ps = out_psum.tile([P, D], f32, tag="op", name="ops")
                        for j in range(JH):
                            nc.tensor.matmul(
                                ps, lhsT=cxt[j][:, :, m * P:(m + 1) * P],
                                rhs=wo_t[j], start=(j == 0), stop=(j == JH - 1),
                                perf_mode=DR)
                        seed = qres_sb[m] if h == 0 else acc_sb[m]
                        nc.vector.scalar_tensor_tensor(
                            out=acc_sb[m], in0=ps, scalar=recip[:, m:m + 1],
                            in1=seed, op0=OP.mult, op1=OP.add)

                # ---- residual bias + LayerNorm + store
                for m in range(MT):
                    x = acc_sb[m]
                    if nonzero_bias:
                        nc.vector.tensor_add(x, x, bo_b)
                    st = stat_pool.tile([P, 6], f32, tag="st", name="st")
                    nc.vector.bn_stats(st, x)
                    mv = stat_pool.tile([P, 2], f32, tag="mv", name="mv")
                    nc.vector.bn_aggr(mv, st)
                    std = stat_pool.tile([P, 1], f32, tag="sd", name="std")
                    nc.scalar.activation(std, mv[:, 1:2], AF.Sqrt, bias=eps_t)
                    rstd = stat_pool.tile([P, 1], f32, tag="rs", name="rstd")
                    nc.vector.reciprocal(rstd, std)
                    y = resid_pool.tile([P, D], f32, tag="resid", name="y")
                    nc.vector.tensor_scalar(
                        out=y, in0=x, scalar1=mv[:, 0:1], scalar2=rstd,
                        op0=OP.subtract, op1=OP.mult)
                    if nonzero_affine:
                        nc.vector.tensor_mul(y, y, gam_b)
                        nc.vector.tensor_add(y, y, bet_b)
                    nc.sync.dma_start(out[m * P:(m + 1) * P, :], y)

            if repeat == 1:
                body()
            else:
                with tc.For_i(0, repeat, 1) as iv:
                    body(iv)

    nc.compile()
    return nc





def _pack_pairs_rows(a):
    """[K*128, C] -> [K//2, 128, 2, C] pair layout (k = 2*j + par, row = k*128+p)."""
    K = a.shape[0] // P
    return np.ascontiguousarray(
        a.reshape(K // 2, 2, P, a.shape[1]).transpose(0, 2, 1, 3))


def _prep_in_maps(inputs):
    f8 = ml_dtypes.float8_e4m3
    Q = np.asarray(inputs["Q"], np.float32)
    K = np.asarray(inputs["K"], np.float32)
    V = np.asarray(inputs["V"], np.float32)
    mask = np.asarray(inputs["attn_mask"])
    wq = _pack_pairs_rows(np.asarray(inputs["Wq"], np.float32)).astype(f8)
    wk = _pack_pairs_rows(np.asarray(inputs["Wk"], np.float32)).astype(f8)
    wv = _pack_pairs_rows(np.asarray(inputs["Wv"], np.float32)).astype(f8)
    # Wo: [H*DH, D] -> [H, JH, 128, 2, D]
    wo = np.asarray(inputs["Wo"], np.float32).reshape(H, JH, 2, P, D)
    wo = np.ascontiguousarray(wo.transpose(0, 1, 3, 2, 4)).astype(f8)

    nonzero_bias = any(np.any(np.asarray(inputs[k])) for k in ("bq", "bk", "bv", "bo"))
    nonzero_affine = (np.any(np.asarray(inputs["gamma"]) != 1.0)
                      or np.any(np.asarray(inputs["beta"])))

    in_maps = []
    for b in range(B):
        m = {
            "xq": _pack_pairs_rows(np.ascontiguousarray(Q[b].T)).astype(f8),
            "xk": _pack_pairs_rows(np.ascontiguousarray(K[b].T)).astype(f8),
            "xv": _pack_pairs_rows(np.ascontiguousarray(V[b].T)).astype(f8),
            "qres": np.ascontiguousarray(Q[b]),
            "maskT": _pack_pairs_rows(np.ascontiguousarray(
                np.where(mask[b].T, np.float32(NEG), np.float32(0))
            )).astype(ml_dtypes.bfloat16),
            "wq": wq, "wk": wk, "wv": wv, "wo": wo,
        }
        if nonzero_bias:
            m["bq"] = np.asarray(inputs["bq"], np.float32)
            m["bk"] = np.asarray(inputs["bk"], np.float32)
            m["bv"] = np.asarray(inputs["bv"], np.float32)
            m["bo"] = np.asarray(inputs["bo"], np.float32)
        if nonzero_affine:
            m["gam"] = np.asarray(inputs["gamma"], np.float32)
            m["bet"] = np.asarray(inputs["beta"], np.float32)
        in_maps.append(m)
    return in_maps, nonzero_bias, nonzero_affine


def kernel(**inputs):
    from concourse.bass_utils import run_bass_kernel_spmd

    in_maps, nzb, nza = _prep_in_maps(inputs)
    key = (1, nzb, nza)
    if key not in _cache:
        _cache[key] = build_nc(repeat=1, nonzero_bias=nzb, nonzero_affine=nza)
    nc = _cache[key]
    res = run_bass_kernel_spmd(nc, in_maps, list(range(B)))
    return np.stack([res.results[c]["out"] for c in range(B)], axis=0).astype(np.float32)
